# revision 23
# baseline (speedup 1.0000x reference)
"""Causal single-head attention on 8 TRN2 NeuronCores (Bass/Tile).

Problem: x[4,4096,1024] @ {Wq,Wk,Wv}[1024,64] (+zero biases) -> causal
softmax attention -> out[4,4096,64], fp32.

Sharding: 8 cores = 4 batches x 2 parities. Each core owns 4 query
blocks of 512 rows. Parity-1 cores receive x^T rolled left by 512
columns so every core's query blocks sit at uniform offsets 1024*i,
keeping the program SPMD-identical; causality is enforced by 4
data-driven diagonal mask tiles plus a parity-dependent pre-exp bias
(-1e30 kills the wrap-around key tiles on parity-0 cores).

Softmax uses no max-subtraction (scores ~N(0,0.25^2), exp is safe) and
the denominator comes from a ones-column appended to V, so there are no
cross-partition reductions. All matmuls run as float32r (full PE rate
at N>=256 moving dim; rel err ~2e-4). The attention inner loop is
software-pipelined (S matmul issued two key-tiles ahead of the AV
accumulation) so the PE does not stall on the ScalarE exp latency.
"""

import numpy as np

B, T, D, H = 4, 4096, 1024, 64
NCORES = 8
QB = 512          # query block width (free dim of attention matmuls)
KT = 128          # key tile (partition dim of P^T)
DC = D // 128     # 8 contraction chunks
CB = 512          # x^T column block for streaming
NCB = T // CB     # 8
NKT = T // KT     # 32
NB = 4            # query blocks per core
HE = H + 1        # V extended with a ones column (softmax denominator)

_PROGRAM = None


def _build_program():
    from contextlib import ExitStack

    import concourse.bass as bass  # noqa: F401
    import concourse.mybir as mybir
    import concourse.tile as tile
    from concourse import bacc
    from concourse.masks import make_identity

    f32 = mybir.dt.float32
    f32r = mybir.dt.float32r
    AF = mybir.ActivationFunctionType

    nc = bacc.Bacc(target_bir_lowering=False)
    xt_d = nc.dram_tensor("xt", [D, T], f32r, kind="ExternalInput").ap()
    wq_d = nc.dram_tensor("wq", [D, H], f32r, kind="ExternalInput").ap()
    wkv_d = nc.dram_tensor("wkv", [D, 2 * H], f32r, kind="ExternalInput").ap()
    bq_d = nc.dram_tensor("bq", [H, 1], f32, kind="ExternalInput").ap()
    bkv_d = nc.dram_tensor("bkv", [2 * H, 1], f32, kind="ExternalInput").ap()
    mk_d = nc.dram_tensor("masks", [4, KT, QB], f32r, kind="ExternalInput").ap()
    tb_d = nc.dram_tensor("tb", [KT, 1], f32, kind="ExternalInput").ap()
    on_d = nc.dram_tensor("ones", [128, NKT], f32r, kind="ExternalInput").ap()
    o_d = nc.dram_tensor("o", [NB * QB, H], f32, kind="ExternalOutput").ap()

    with ExitStack() as ctx:
        tc = ctx.enter_context(tile.TileContext(nc))
        const = ctx.enter_context(tc.tile_pool(name="const", bufs=1))
        xt_pool = ctx.enter_context(tc.tile_pool(name="xtp", bufs=5))
        ppool = ctx.enter_context(tc.tile_pool(name="ptp", bufs=4))
        opool = ctx.enter_context(tc.tile_pool(name="otp", bufs=2))
        ps_a = ctx.enter_context(tc.tile_pool(name="psA", bufs=1, space="PSUM"))
        ps_s = ctx.enter_context(tc.tile_pool(name="psS", bufs=3, space="PSUM"))
        ps_o = ctx.enter_context(tc.tile_pool(name="psO", bufs=1, space="PSUM"))
        ps_t = ctx.enter_context(tc.tile_pool(name="psT", bufs=2, space="PSUM"))

        # Persistent SBUF state
        wq_s = const.tile([128, DC * H], f32r)        # chunk d at cols d*H
        wkv_s = const.tile([128, DC * 2 * H], f32r)   # chunk d at cols d*2H
        bq_s = const.tile([H, 1], f32)
        bkv_s = const.tile([2 * H, 1], f32)
        mk_s = const.tile([KT, 4 * QB], f32r)         # mask slot s at cols s*QB
        tb_s = const.tile([KT, 1], f32)              # tail-tile exp bias
        zb_s = const.tile([KT, 1], f32)              # zero exp bias
        nc.vector.memset(zb_s, 0.0)
        ident = const.tile([128, 128], f32)
        kv_s = const.tile([128, T], f32r)             # rows 0:64 k^T, 64:128 v^T
        ve_s = const.tile([128, NKT * HE], f32r)      # key tile j at cols j*HE
        qt_s = const.tile([H, NB * QB], f32r)         # q^T, block i at cols i*QB

        make_identity(nc, ident)
        nc.sync.dma_start(
            out=wkv_s.rearrange("p (d h) -> p d h", d=DC),
            in_=wkv_d.rearrange("(d p) h -> p d h", p=128),
        )
        nc.sync.dma_start(
            out=wq_s.rearrange("p (d h) -> p d h", d=DC),
            in_=wq_d.rearrange("(d p) h -> p d h", p=128),
        )
        nc.sync.dma_start(out=bq_s, in_=bq_d)
        nc.sync.dma_start(out=bkv_s, in_=bkv_d)
        nc.sync.dma_start(out=tb_s, in_=tb_d)

        def load_consts_late():
            # Emitted after the first two x^T col-block DMAs so the big
            # mask transfer doesn't delay the first projection matmuls.
            nc.sync.dma_start(
                out=mk_s.rearrange("p (s c) -> p s c", s=4),
                in_=mk_d.rearrange("s p c -> p s c"),
            )
            # Ones column of extended V (softmax denominator), strided into
            # every key tile's column H. memset can't target f32r tiles, so
            # the ones come from a tiny DRAM input.
            nc.sync.dma_start(
                out=ve_s.rearrange("p (j e) -> p j e", e=HE)[:, :, H:H + 1],
                in_=on_d.rearrange("p (j e) -> p j e", e=1),
            )

        def stage_a(t, first=False):
            """Stream x^T col-block t; project K/V (and Q on even t)."""
            xt_t = xt_pool.tile([128, DC * CB], f32r)  # chunk d at cols d*CB
            nc.sync.dma_start(
                out=xt_t.rearrange("p (d c) -> p d c", d=DC),
                in_=xt_d.rearrange("(d p) t -> p d t", p=128)[
                    :, :, t * CB:(t + 1) * CB
                ],
            )
            pkv = ps_a.tile([128, CB], f32, tag="pkv")
            for d in range(DC):
                nc.tensor.matmul(
                    pkv,
                    lhsT=wkv_s[:, d * 128:(d + 1) * 128],
                    rhs=xt_t[:, d * CB:(d + 1) * CB],
                    start=(d == 0),
                    stop=(d == DC - 1),
                )
            pq = None
            if t % 2 == 0:
                # Q projection MMs right after KV's keep PE busy while DVE
                # drains the KV psum.
                pq = ps_a.tile([H, CB], f32, tag="pq")
                for d in range(DC):
                    nc.tensor.matmul(
                        pq,
                        lhsT=wq_s[:, d * H:(d + 1) * H],
                        rhs=xt_t[:, d * CB:(d + 1) * CB],
                        start=(d == 0),
                        stop=(d == DC - 1),
                    )
            nc.vector.tensor_scalar_add(
                kv_s[:, t * CB:(t + 1) * CB], pkv, bkv_s
            )
            if pq is not None:
                i = t // 2
                nc.vector.tensor_scalar_add(
                    qt_s[:, i * QB:(i + 1) * QB], pq, bq_s
                )
            for sub in range(4):                     # v^T -> natural-v tiles
                j = 4 * t + sub
                ptr = ps_t.tile([128, HE], f32, tag="tr")
                nc.tensor.transpose(
                    ptr[:, 0:H],
                    kv_s[64:128, t * CB + sub * KT:t * CB + (sub + 1) * KT].bitcast(f32),
                    ident[64:128, 64:128],
                )
                nc.vector.tensor_copy(ve_s[:, j * HE:j * HE + H], ptr[:, 0:H])

        def attn_loop(i, pre_hook=None, mid_hooks=()):
            """Attention for query block i (flash-style, S^T layout).

            Software-pipelined: the S matmul for tile j+2 is issued before
            the AV matmul for tile j, so the PE never stalls on the exp
            (ScalarE) latency of the current tile.
            """
            po = ps_o.tile([HE, QB], f32)
            js = (list(range(8 * i)) + [28, 29, 30, 31]
                  + list(range(8 * i, 8 * i + 4)))
            n = len(js)
            pts = {}

            def emit_s(idx):
                j = js[idx]
                ps = ps_s.tile([KT, QB], f32)
                nc.tensor.matmul(
                    ps,
                    lhsT=kv_s[0:64, j * KT:(j + 1) * KT],
                    rhs=qt_s[:, i * QB:(i + 1) * QB],
                    start=True,
                    stop=True,
                )
                pt = ppool.tile([KT, QB], f32r)
                # Tail (wrap-around) tiles: parity-0 cores kill them with a
                # -1e30 pre-exp bias; parity-1 keeps them (bias 0).
                bias = tb_s if j >= 28 else zb_s
                nc.scalar.activation(
                    pt, ps, AF.Exp, bias=bias, scale=float(D) ** -0.5
                )
                if 8 * i <= j < 8 * i + 4:
                    slot = j - 8 * i                 # diagonal masks
                    nc.vector.tensor_mul(
                        pt, pt, mk_s[:, slot * QB:(slot + 1) * QB]
                    )
                pts[idx] = pt

            emit_s(0)
            emit_s(1)
            if pre_hook is not None:
                pre_hook()                           # previous block epilogue
            hooks = {max(1, (k + 1) * n // (len(mid_hooks) + 1)): h
                     for k, h in enumerate(mid_hooks)}
            for idx in range(n):
                if idx + 2 < n:
                    emit_s(idx + 2)
                nc.tensor.matmul(
                    po,
                    lhsT=ve_s[:, js[idx] * HE:(js[idx] + 1) * HE],
                    rhs=pts.pop(idx),
                    start=(idx == 0),
                    stop=(idx == n - 1),
                )
                if idx in hooks:
                    # Interleave the next x^T col-block's projections into
                    # the attention stream: PE does them while ScalarE
                    # drains the exp backlog.
                    hooks[idx]()
            return po

        def attn_epi(i, po):
            ot_t = opool.tile([HE, QB], f32, tag="oT")
            nc.vector.tensor_copy(ot_t, po)
            for sub in range(4):
                ptr = ps_t.tile([128, HE], f32, tag="tr")
                nc.tensor.transpose(
                    ptr, ot_t[:, sub * 128:(sub + 1) * 128], ident[0:HE, 0:HE]
                )
                rcp = opool.tile([128, 1], f32, tag="rcp")
                nc.vector.reciprocal(rcp, ptr[:, H:H + 1])
                ot = opool.tile([128, H], f32, tag="out")
                nc.vector.tensor_scalar_mul(ot, ptr[:, 0:H], rcp)
                nc.sync.dma_start(
                    out=o_d[i * QB + sub * 128:i * QB + (sub + 1) * 128, :],
                    in_=ot,
                )

        # Col-block 7 first: the wrap-around key tiles 28..31 feed every
        # attention block, so they must land before attn(0). Each block's
        # epilogue is deferred into the next block's stream so its PE
        # transposes hide behind fresh S matmuls.
        stage_a(7, first=True)
        stage_a(0)
        load_consts_late()
        po0 = attn_loop(0)
        stage_a(1)
        stage_a(2)
        po1 = attn_loop(1, pre_hook=lambda: attn_epi(0, po0))
        stage_a(3)
        stage_a(4)
        po2 = attn_loop(2, pre_hook=lambda: attn_epi(1, po1))
        stage_a(5)
        stage_a(6)
        po3 = attn_loop(3, pre_hook=lambda: attn_epi(2, po2))
        attn_epi(3, po3)

    nc.compile()
    return nc


def _get_program():
    global _PROGRAM
    if _PROGRAM is None:
        _PROGRAM = _build_program()
    return _PROGRAM


def _build_masks(p):
    m = np.zeros((4, KT, QB), np.float32)
    pk = np.arange(KT)[:, None]
    c = np.arange(QB)[None, :]
    for s in range(4):
        m[s] = (c >= pk + 128 * s).astype(np.float32)
    return m


def build_in_maps(inputs):
    x = np.asarray(inputs["x"], np.float32)
    wq = np.ascontiguousarray(np.asarray(inputs["Wq"], np.float32))
    wkv = np.ascontiguousarray(
        np.concatenate(
            [np.asarray(inputs["Wk"], np.float32),
             np.asarray(inputs["Wv"], np.float32)], axis=1
        )
    )
    bq = np.ascontiguousarray(np.asarray(inputs["bq"], np.float32)[:, None])
    bkv = np.ascontiguousarray(
        np.concatenate(
            [np.asarray(inputs["bk"], np.float32),
             np.asarray(inputs["bv"], np.float32)]
        )[:, None]
    )
    in_maps = []
    for core in range(NCORES):
        b, p = core // 2, core % 2
        xt = x[b].T
        if p:
            xt = np.roll(xt, -512, axis=1)
        in_maps.append({
            "xt": np.ascontiguousarray(xt),
            "wq": wq,
            "wkv": wkv,
            "bq": bq,
            "bkv": bkv,
            "masks": _build_masks(p),
            "ones": np.ones((128, NKT), np.float32),
            "tb": np.full((KT, 1), 0.0 if p == 1 else -1e30, np.float32),
        })
    return in_maps


def assemble_out(results):
    out = np.empty((B, T, H), np.float32)
    for core in range(NCORES):
        b, p = core // 2, core % 2
        o = np.asarray(results[core]["o"])
        for i in range(NB):
            g = 1024 * i + 512 * p
            out[b, g:g + QB] = o[i * QB:(i + 1) * QB]
    return out


def kernel(**inputs):
    from concourse.bass_utils import run_bass_kernel_spmd

    nc = _get_program()
    in_maps = build_in_maps(inputs)
    res = run_bass_kernel_spmd(nc, in_maps, list(range(NCORES)))
    return assemble_out(res.results)


# revision 25
# speedup vs baseline: 1.0775x; 1.0775x over previous
"""Causal single-head attention on 8 TRN2 NeuronCores (Bass/Tile).

Problem: x[4,4096,1024] @ {Wq,Wk,Wv}[1024,64] (+zero biases) -> causal
softmax attention -> out[4,4096,64], fp32.

Sharding: 8 cores = 4 batches x 2 parities. Each core owns 4 query
blocks of 512 rows. Parity-1 cores receive x^T rolled left by 512
columns so every core's query blocks sit at uniform offsets 1024*i,
keeping the program SPMD-identical; causality is enforced by 4
data-driven diagonal mask tiles plus a parity-dependent pre-exp bias
(-1e30 kills the wrap-around key tiles on parity-0 cores).

Softmax uses no max-subtraction (scores ~N(0,0.25^2), exp is safe) and
the denominator comes from a ones-column appended to V, so there are no
cross-partition reductions. All matmuls run as float32r (full PE rate
at N>=256 moving dim; rel err ~2e-4). The attention inner loop is
software-pipelined (S matmul issued two key-tiles ahead of the AV
accumulation) so the PE does not stall on the ScalarE exp latency.
"""

import numpy as np

B, T, D, H = 4, 4096, 1024, 64
NCORES = 8
QB = 512          # query block width (free dim of attention matmuls)
KT = 128          # key tile (partition dim of P^T)
DC = D // 128     # 8 contraction chunks
CB = 512          # x^T column block for streaming
NCB = T // CB     # 8
NKT = T // KT     # 32
NB = 4            # query blocks per core
HE = H + 1        # V extended with a ones column (softmax denominator)

_PROGRAM = None


def _build_program():
    from contextlib import ExitStack

    import concourse.bass as bass  # noqa: F401
    import concourse.mybir as mybir
    import concourse.tile as tile
    from concourse import bacc
    from concourse.masks import make_identity

    f32 = mybir.dt.float32
    f32r = mybir.dt.float32r
    AF = mybir.ActivationFunctionType

    nc = bacc.Bacc(target_bir_lowering=False)
    xt_d = nc.dram_tensor("xt", [D, T], f32r, kind="ExternalInput").ap()
    wq_d = nc.dram_tensor("wq", [D, H], f32r, kind="ExternalInput").ap()
    wkv_d = nc.dram_tensor("wkv", [D, 2 * H], f32r, kind="ExternalInput").ap()
    bq_d = nc.dram_tensor("bq", [H, 1], f32, kind="ExternalInput").ap()
    bkv_d = nc.dram_tensor("bkv", [2 * H, 1], f32, kind="ExternalInput").ap()
    mk_d = nc.dram_tensor("masks", [4, KT, QB], f32r, kind="ExternalInput").ap()
    tb_d = nc.dram_tensor("tb", [KT, 1], f32, kind="ExternalInput").ap()
    on_d = nc.dram_tensor("ones", [128, NKT], f32r, kind="ExternalInput").ap()
    o_d = nc.dram_tensor("o", [NB * QB, H], f32, kind="ExternalOutput").ap()

    with ExitStack() as ctx:
        tc = ctx.enter_context(tile.TileContext(nc))
        const = ctx.enter_context(tc.tile_pool(name="const", bufs=1))
        xt_pool = ctx.enter_context(tc.tile_pool(name="xtp", bufs=5))
        ppool = ctx.enter_context(tc.tile_pool(name="ptp", bufs=4))
        opool = ctx.enter_context(tc.tile_pool(name="otp", bufs=2))
        ps_a = ctx.enter_context(tc.tile_pool(name="psA", bufs=1, space="PSUM"))
        ps_s = ctx.enter_context(tc.tile_pool(name="psS", bufs=3, space="PSUM"))
        ps_o = ctx.enter_context(tc.tile_pool(name="psO", bufs=1, space="PSUM"))
        ps_t = ctx.enter_context(tc.tile_pool(name="psT", bufs=2, space="PSUM"))

        # Persistent SBUF state
        wq_s = const.tile([128, DC * H], f32r)        # chunk d at cols d*H
        wkv_s = const.tile([128, DC * 2 * H], f32r)   # chunk d at cols d*2H
        bq_s = const.tile([H, 1], f32)
        bkv_s = const.tile([2 * H, 1], f32)
        mk_s = const.tile([KT, 4 * QB], f32r)         # mask slot s at cols s*QB
        tb_s = const.tile([KT, 1], f32)              # tail-tile exp bias
        zb_s = const.tile([KT, 1], f32)              # zero exp bias
        nc.vector.memset(zb_s, 0.0)
        ident = const.tile([128, 128], f32)
        kv_s = const.tile([128, T], f32r)             # rows 0:64 k^T, 64:128 v^T
        ve_s = const.tile([128, NKT * HE], f32r)      # key tile j at cols j*HE
        qt_s = const.tile([H, NB * QB], f32r)         # q^T, block i at cols i*QB

        make_identity(nc, ident)
        nc.sync.dma_start(
            out=wkv_s.rearrange("p (d h) -> p d h", d=DC),
            in_=wkv_d.rearrange("(d p) h -> p d h", p=128),
        )
        nc.sync.dma_start(
            out=wq_s.rearrange("p (d h) -> p d h", d=DC),
            in_=wq_d.rearrange("(d p) h -> p d h", p=128),
        )
        nc.sync.dma_start(out=bq_s, in_=bq_d)
        nc.sync.dma_start(out=bkv_s, in_=bkv_d)
        nc.sync.dma_start(out=tb_s, in_=tb_d)

        def load_consts_late():
            # Emitted after the first two x^T col-block DMAs so the big
            # mask transfer doesn't delay the first projection matmuls.
            nc.sync.dma_start(
                out=mk_s.rearrange("p (s c) -> p s c", s=4),
                in_=mk_d.rearrange("s p c -> p s c"),
            )
            # Ones column of extended V (softmax denominator), strided into
            # every key tile's column H. memset can't target f32r tiles, so
            # the ones come from a tiny DRAM input.
            nc.sync.dma_start(
                out=ve_s.rearrange("p (j e) -> p j e", e=HE)[:, :, H:H + 1],
                in_=on_d.rearrange("p (j e) -> p j e", e=1),
            )

        def stage_a(t, first=False):
            """Stream x^T col-block t; project K/V (and Q on even t)."""
            xt_t = xt_pool.tile([128, DC * CB], f32r)  # chunk d at cols d*CB
            nc.sync.dma_start(
                out=xt_t.rearrange("p (d c) -> p d c", d=DC),
                in_=xt_d.rearrange("(d p) t -> p d t", p=128)[
                    :, :, t * CB:(t + 1) * CB
                ],
            )
            pkv = ps_a.tile([128, CB], f32, tag="pkv")
            for d in range(DC):
                nc.tensor.matmul(
                    pkv,
                    lhsT=wkv_s[:, d * 128:(d + 1) * 128],
                    rhs=xt_t[:, d * CB:(d + 1) * CB],
                    start=(d == 0),
                    stop=(d == DC - 1),
                )
            pq = None
            if t % 2 == 0:
                # Q projection MMs right after KV's keep PE busy while DVE
                # drains the KV psum.
                pq = ps_a.tile([H, CB], f32, tag="pq")
                for d in range(DC):
                    nc.tensor.matmul(
                        pq,
                        lhsT=wq_s[:, d * H:(d + 1) * H],
                        rhs=xt_t[:, d * CB:(d + 1) * CB],
                        start=(d == 0),
                        stop=(d == DC - 1),
                    )
            nc.vector.tensor_scalar_add(
                kv_s[:, t * CB:(t + 1) * CB], pkv, bkv_s
            )
            if pq is not None:
                i = t // 2
                nc.vector.tensor_scalar_add(
                    qt_s[:, i * QB:(i + 1) * QB], pq, bq_s
                )
            for sub in range(4):                     # v^T -> natural-v tiles
                j = 4 * t + sub
                ptr = ps_t.tile([128, HE], f32, tag="tr")
                nc.tensor.transpose(
                    ptr[:, 0:H],
                    kv_s[64:128, t * CB + sub * KT:t * CB + (sub + 1) * KT].bitcast(f32),
                    ident[64:128, 64:128],
                )
                nc.vector.tensor_copy(ve_s[:, j * HE:j * HE + H], ptr[:, 0:H])

        # Per-block SBUF accumulators for (PV | denom)^T; pieces of a
        # block's key loop flush their PSUM partial here so attention can
        # be emitted piecewise as kv col-blocks arrive.
        oacc = []
        for _i in range(NB):
            acc_tile = const.tile([HE, QB], f32, tag=f"oacc{_i}")
            oacc.append(acc_tile)
        first_piece = [True] * NB

        def attn_piece(i, js):
            """Emit S->exp->mask->AV for the given key tiles of block i.

            Software-pipelined: the S matmul for tile idx+2 is issued
            before the AV matmul for tile idx, so the PE does not stall on
            the exp (ScalarE) latency. The piece's PSUM partial is added
            into the block's SBUF accumulator.
            """
            po = ps_o.tile([HE, QB], f32)
            n = len(js)
            pts = {}

            def emit_s(idx):
                j = js[idx]
                ps = ps_s.tile([KT, QB], f32)
                nc.tensor.matmul(
                    ps,
                    lhsT=kv_s[0:64, j * KT:(j + 1) * KT],
                    rhs=qt_s[:, i * QB:(i + 1) * QB],
                    start=True,
                    stop=True,
                )
                pt = ppool.tile([KT, QB], f32r)
                # Tail (wrap-around) tiles: parity-0 cores kill them with a
                # -1e30 pre-exp bias; parity-1 keeps them (bias 0).
                bias = tb_s if j >= 28 else zb_s
                nc.scalar.activation(
                    pt, ps, AF.Exp, bias=bias, scale=float(D) ** -0.5
                )
                if 8 * i <= j < 8 * i + 4:
                    slot = j - 8 * i                 # diagonal masks
                    nc.vector.tensor_mul(
                        pt, pt, mk_s[:, slot * QB:(slot + 1) * QB]
                    )
                pts[idx] = pt

            emit_s(0)
            if n > 1:
                emit_s(1)
            for idx in range(n):
                if idx + 2 < n:
                    emit_s(idx + 2)
                nc.tensor.matmul(
                    po,
                    lhsT=ve_s[:, js[idx] * HE:(js[idx] + 1) * HE],
                    rhs=pts.pop(idx),
                    start=(idx == 0),
                    stop=(idx == n - 1),
                )
            if first_piece[i]:
                nc.vector.tensor_copy(oacc[i], po)
                first_piece[i] = False
            else:
                nc.vector.tensor_add(oacc[i], oacc[i], po)

        def attn_epi(i):
            for sub in range(4):
                ptr = ps_t.tile([128, HE], f32, tag="tr")
                nc.tensor.transpose(
                    ptr, oacc[i][:, sub * 128:(sub + 1) * 128],
                    ident[0:HE, 0:HE]
                )
                rcp = opool.tile([128, 1], f32, tag="rcp")
                nc.vector.reciprocal(rcp, ptr[:, H:H + 1])
                ot = opool.tile([128, H], f32, tag="out")
                nc.vector.tensor_scalar_mul(ot, ptr[:, 0:H], rcp)
                nc.sync.dma_start(
                    out=o_d[i * QB + sub * 128:i * QB + (sub + 1) * 128, :],
                    in_=ot,
                )

        # Evens-first streaming: col-block 7 first (the wrap-around key
        # tiles feed every block's tail), then even col-blocks (which hold
        # all four query blocks), then odds. Each block's attention is
        # emitted piecewise the moment its kv dependencies are resident,
        # so the exp stream never waits behind a later DMA in PE order.
        # Piece deps: tails j>=28 -> kv(7); diag j in [8i,8i+4) -> col
        # block 2i; full j -> col-block j//4.
        stage_a(7, first=True)
        stage_a(0)
        load_consts_late()
        attn_piece(0, [28, 29, 30, 31] + [0, 1, 2, 3])        # tails+diag
        attn_epi(0)
        stage_a(2)
        attn_piece(1, [28, 29, 30, 31] + [0, 1, 2, 3] + [8, 9, 10, 11])
        stage_a(4)
        attn_piece(2, [28, 29, 30, 31] + [0, 1, 2, 3]
                   + [8, 9, 10, 11] + [16, 17, 18, 19])
        stage_a(6)
        attn_piece(3, [28, 29, 30, 31] + [0, 1, 2, 3] + [8, 9, 10, 11]
                   + [16, 17, 18, 19] + [24, 25, 26, 27])
        stage_a(1)
        attn_piece(1, [4, 5, 6, 7])
        attn_epi(1)
        attn_piece(2, [4, 5, 6, 7])
        attn_piece(3, [4, 5, 6, 7])
        stage_a(3)
        attn_piece(2, [12, 13, 14, 15])
        attn_epi(2)
        attn_piece(3, [12, 13, 14, 15])
        stage_a(5)
        attn_piece(3, [20, 21, 22, 23])
        attn_epi(3)

    nc.compile()
    return nc


def _get_program():
    global _PROGRAM
    if _PROGRAM is None:
        _PROGRAM = _build_program()
    return _PROGRAM


def _build_masks(p):
    m = np.zeros((4, KT, QB), np.float32)
    pk = np.arange(KT)[:, None]
    c = np.arange(QB)[None, :]
    for s in range(4):
        m[s] = (c >= pk + 128 * s).astype(np.float32)
    return m


def build_in_maps(inputs):
    x = np.asarray(inputs["x"], np.float32)
    wq = np.ascontiguousarray(np.asarray(inputs["Wq"], np.float32))
    wkv = np.ascontiguousarray(
        np.concatenate(
            [np.asarray(inputs["Wk"], np.float32),
             np.asarray(inputs["Wv"], np.float32)], axis=1
        )
    )
    bq = np.ascontiguousarray(np.asarray(inputs["bq"], np.float32)[:, None])
    bkv = np.ascontiguousarray(
        np.concatenate(
            [np.asarray(inputs["bk"], np.float32),
             np.asarray(inputs["bv"], np.float32)]
        )[:, None]
    )
    in_maps = []
    for core in range(NCORES):
        b, p = core // 2, core % 2
        xt = x[b].T
        if p:
            xt = np.roll(xt, -512, axis=1)
        in_maps.append({
            "xt": np.ascontiguousarray(xt),
            "wq": wq,
            "wkv": wkv,
            "bq": bq,
            "bkv": bkv,
            "masks": _build_masks(p),
            "ones": np.ones((128, NKT), np.float32),
            "tb": np.full((KT, 1), 0.0 if p == 1 else -1e30, np.float32),
        })
    return in_maps


def assemble_out(results):
    out = np.empty((B, T, H), np.float32)
    for core in range(NCORES):
        b, p = core // 2, core % 2
        o = np.asarray(results[core]["o"])
        for i in range(NB):
            g = 1024 * i + 512 * p
            out[b, g:g + QB] = o[i * QB:(i + 1) * QB]
    return out


def kernel(**inputs):
    from concourse.bass_utils import run_bass_kernel_spmd

    nc = _get_program()
    in_maps = build_in_maps(inputs)
    res = run_bass_kernel_spmd(nc, in_maps, list(range(NCORES)))
    return assemble_out(res.results)


# revision 28
# speedup vs baseline: 1.0802x; 1.0025x over previous
"""Causal single-head attention on 8 TRN2 NeuronCores (Bass/Tile).

Problem: x[4,4096,1024] @ {Wq,Wk,Wv}[1024,64] (+zero biases) -> causal
softmax attention -> out[4,4096,64], fp32.

Sharding: 8 cores = 4 batches x 2 parities. Each core owns 4 query
blocks of 512 rows. Parity-1 cores receive x^T rolled left by 512
columns so every core's query blocks sit at uniform offsets 1024*i,
keeping the program SPMD-identical; causality is enforced by 4
data-driven diagonal mask tiles plus a parity-dependent pre-exp bias
(-1e30 kills the wrap-around key tiles on parity-0 cores).

Softmax uses no max-subtraction (scores ~N(0,0.25^2), exp is safe) and
the denominator comes from a ones-column appended to V, so there are no
cross-partition reductions. All matmuls run as float32r (full PE rate
at N>=256 moving dim; rel err ~2e-4). The attention inner loop is
software-pipelined (S matmul issued two key-tiles ahead of the AV
accumulation) so the PE does not stall on the ScalarE exp latency.
"""

import numpy as np

B, T, D, H = 4, 4096, 1024, 64
NCORES = 8
QB = 512          # query block width (free dim of attention matmuls)
KT = 128          # key tile (partition dim of P^T)
DC = D // 128     # 8 contraction chunks
CB = 512          # x^T column block for streaming
NCB = T // CB     # 8
NKT = T // KT     # 32
NB = 4            # query blocks per core
HE = H + 1        # V extended with a ones column (softmax denominator)

_PROGRAM = None


def _build_program():
    from contextlib import ExitStack

    import concourse.bass as bass  # noqa: F401
    import concourse.mybir as mybir
    import concourse.tile as tile
    from concourse import bacc
    from concourse.masks import make_identity

    f32 = mybir.dt.float32
    f32r = mybir.dt.float32r
    AF = mybir.ActivationFunctionType

    nc = bacc.Bacc(target_bir_lowering=False)
    xt_d = nc.dram_tensor("xt", [D, T], f32r, kind="ExternalInput").ap()
    wq_d = nc.dram_tensor("wq", [D, H], f32r, kind="ExternalInput").ap()
    wkv_d = nc.dram_tensor("wkv", [D, 2 * H], f32r, kind="ExternalInput").ap()
    bq_d = nc.dram_tensor("bq", [H, 1], f32, kind="ExternalInput").ap()
    bkv_d = nc.dram_tensor("bkv", [2 * H, 1], f32, kind="ExternalInput").ap()
    mk_d = nc.dram_tensor("masks", [4, KT, QB], f32r, kind="ExternalInput").ap()
    tb_d = nc.dram_tensor("tb", [KT, 1], f32, kind="ExternalInput").ap()
    on_d = nc.dram_tensor("ones", [128, NKT], f32r, kind="ExternalInput").ap()
    o_d = nc.dram_tensor("o", [NB * QB, H], f32, kind="ExternalOutput").ap()

    with ExitStack() as ctx:
        tc = ctx.enter_context(tile.TileContext(nc))
        const = ctx.enter_context(tc.tile_pool(name="const", bufs=1))
        xt_pool = ctx.enter_context(tc.tile_pool(name="xtp", bufs=5))
        ppool = ctx.enter_context(tc.tile_pool(name="ptp", bufs=6))
        opool = ctx.enter_context(tc.tile_pool(name="otp", bufs=2))
        ps_a = ctx.enter_context(tc.tile_pool(name="psA", bufs=1, space="PSUM"))
        ps_s = ctx.enter_context(tc.tile_pool(name="psS", bufs=3, space="PSUM"))
        ps_o = ctx.enter_context(tc.tile_pool(name="psO", bufs=1, space="PSUM"))
        ps_t = ctx.enter_context(tc.tile_pool(name="psT", bufs=2, space="PSUM"))

        # Persistent SBUF state
        wq_s = const.tile([128, DC * H], f32r)        # chunk d at cols d*H
        wkv_s = const.tile([128, DC * 2 * H], f32r)   # chunk d at cols d*2H
        bq_s = const.tile([H, 1], f32)
        bkv_s = const.tile([2 * H, 1], f32)
        mk_s = const.tile([KT, 4 * QB], f32r)         # mask slot s at cols s*QB
        tb_s = const.tile([KT, 1], f32)              # tail-tile exp bias
        zb_s = const.tile([KT, 1], f32)              # zero exp bias
        nc.vector.memset(zb_s, 0.0)
        ident = const.tile([128, 128], f32)
        kv_s = const.tile([128, T], f32r)             # rows 0:64 k^T, 64:128 v^T
        ve_s = const.tile([128, NKT * HE], f32r)      # key tile j at cols j*HE
        qt_s = const.tile([H, NB * QB], f32r)         # q^T, block i at cols i*QB

        make_identity(nc, ident)
        nc.sync.dma_start(
            out=wkv_s.rearrange("p (d h) -> p d h", d=DC),
            in_=wkv_d.rearrange("(d p) h -> p d h", p=128),
        )
        nc.sync.dma_start(
            out=wq_s.rearrange("p (d h) -> p d h", d=DC),
            in_=wq_d.rearrange("(d p) h -> p d h", p=128),
        )
        nc.sync.dma_start(out=bq_s, in_=bq_d)
        nc.sync.dma_start(out=bkv_s, in_=bkv_d)
        nc.sync.dma_start(out=tb_s, in_=tb_d)

        def load_consts_late():
            # Emitted after the first two x^T col-block DMAs so the big
            # mask transfer doesn't delay the first projection matmuls.
            nc.sync.dma_start(
                out=mk_s.rearrange("p (s c) -> p s c", s=4),
                in_=mk_d.rearrange("s p c -> p s c"),
            )
            # Ones column of extended V (softmax denominator), strided into
            # every key tile's column H. memset can't target f32r tiles, so
            # the ones come from a tiny DRAM input.
            nc.sync.dma_start(
                out=ve_s.rearrange("p (j e) -> p j e", e=HE)[:, :, H:H + 1],
                in_=on_d.rearrange("p (j e) -> p j e", e=1),
            )

        def stage_a(t, first=False):
            """Stream x^T col-block t; project K/V (and Q on even t)."""
            xt_t = xt_pool.tile([128, DC * CB], f32r)  # chunk d at cols d*CB
            nc.sync.dma_start(
                out=xt_t.rearrange("p (d c) -> p d c", d=DC),
                in_=xt_d.rearrange("(d p) t -> p d t", p=128)[
                    :, :, t * CB:(t + 1) * CB
                ],
            )
            pkv = ps_a.tile([128, CB], f32, tag="pkv")
            for d in range(DC):
                nc.tensor.matmul(
                    pkv,
                    lhsT=wkv_s[:, d * 128:(d + 1) * 128],
                    rhs=xt_t[:, d * CB:(d + 1) * CB],
                    start=(d == 0),
                    stop=(d == DC - 1),
                )
            pq = None
            if t % 2 == 0:
                # Q projection MMs right after KV's keep PE busy while DVE
                # drains the KV psum.
                pq = ps_a.tile([H, CB], f32, tag="pq")
                for d in range(DC):
                    nc.tensor.matmul(
                        pq,
                        lhsT=wq_s[:, d * H:(d + 1) * H],
                        rhs=xt_t[:, d * CB:(d + 1) * CB],
                        start=(d == 0),
                        stop=(d == DC - 1),
                    )
            nc.vector.tensor_scalar_add(
                kv_s[:, t * CB:(t + 1) * CB], pkv, bkv_s
            )
            if pq is not None:
                i = t // 2
                nc.vector.tensor_scalar_add(
                    qt_s[:, i * QB:(i + 1) * QB], pq, bq_s
                )
            for sub in range(4):                     # v^T -> natural-v tiles
                j = 4 * t + sub
                ptr = ps_t.tile([128, HE], f32, tag="tr")
                nc.tensor.transpose(
                    ptr[:, 0:H],
                    kv_s[64:128, t * CB + sub * KT:t * CB + (sub + 1) * KT].bitcast(f32),
                    ident[64:128, 64:128],
                )
                nc.vector.tensor_copy(ve_s[:, j * HE:j * HE + H], ptr[:, 0:H])

        # Per-block SBUF accumulators for (PV | denom)^T; pieces of a
        # block's key loop flush their PSUM partial here so attention can
        # be emitted piecewise as kv col-blocks arrive.
        oacc = []
        for _i in range(NB):
            acc_tile = const.tile([HE, QB], f32, tag=f"oacc{_i}")
            oacc.append(acc_tile)
        first_piece = [True] * NB

        def attn_piece(i, js):
            """Emit S->exp->mask->AV for the given key tiles of block i.

            Software-pipelined: the S matmul for tile idx+2 is issued
            before the AV matmul for tile idx, so the PE does not stall on
            the exp (ScalarE) latency. The piece's PSUM partial is added
            into the block's SBUF accumulator.
            """
            po = ps_o.tile([HE, QB], f32)
            n = len(js)
            pts = {}

            def emit_s(idx):
                j = js[idx]
                ps = ps_s.tile([KT, QB], f32)
                nc.tensor.matmul(
                    ps,
                    lhsT=kv_s[0:64, j * KT:(j + 1) * KT],
                    rhs=qt_s[:, i * QB:(i + 1) * QB],
                    start=True,
                    stop=True,
                )
                pt = ppool.tile([KT, QB], f32r)
                # Tail (wrap-around) tiles: parity-0 cores kill them with a
                # -1e30 pre-exp bias; parity-1 keeps them (bias 0).
                bias = tb_s if j >= 28 else zb_s
                nc.scalar.activation(
                    pt, ps, AF.Exp, bias=bias, scale=float(D) ** -0.5
                )
                if 8 * i <= j < 8 * i + 4:
                    slot = j - 8 * i                 # diagonal masks
                    nc.vector.tensor_mul(
                        pt, pt, mk_s[:, slot * QB:(slot + 1) * QB]
                    )
                pts[idx] = pt

            emit_s(0)
            if n > 1:
                emit_s(1)
            for idx in range(n):
                if idx + 2 < n:
                    emit_s(idx + 2)
                nc.tensor.matmul(
                    po,
                    lhsT=ve_s[:, js[idx] * HE:(js[idx] + 1) * HE],
                    rhs=pts.pop(idx),
                    start=(idx == 0),
                    stop=(idx == n - 1),
                )
            if first_piece[i]:
                nc.vector.tensor_copy(oacc[i], po)
                first_piece[i] = False
            else:
                nc.vector.tensor_add(oacc[i], oacc[i], po)

        def attn_epi(i):
            for sub in range(4):
                ptr = ps_t.tile([128, HE], f32, tag="tr")
                nc.tensor.transpose(
                    ptr, oacc[i][:, sub * 128:(sub + 1) * 128],
                    ident[0:HE, 0:HE]
                )
                rcp = opool.tile([128, 1], f32, tag="rcp")
                nc.vector.reciprocal(rcp, ptr[:, H:H + 1])
                ot = opool.tile([128, H], f32, tag="out")
                nc.vector.tensor_scalar_mul(ot, ptr[:, 0:H], rcp)
                nc.sync.dma_start(
                    out=o_d[i * QB + sub * 128:i * QB + (sub + 1) * 128, :],
                    in_=ot,
                )

        # Evens-first streaming: col-block 7 first (the wrap-around key
        # tiles feed every block's tail), then even col-blocks (which hold
        # all four query blocks), then odds. Each block's attention is
        # emitted piecewise the moment its kv dependencies are resident,
        # so the exp stream never waits behind a later DMA in PE order.
        # Piece deps: tails j>=28 -> kv(7); diag j in [8i,8i+4) -> col
        # block 2i; full j -> col-block j//4.
        stage_a(7, first=True)
        stage_a(0)
        load_consts_late()
        attn_piece(0, [28, 29, 30, 31] + [0, 1, 2, 3])        # tails+diag
        attn_epi(0)
        stage_a(2)
        attn_piece(1, [28, 29, 30, 31] + [0, 1, 2, 3] + [8, 9, 10, 11])
        stage_a(4)
        attn_piece(2, [28, 29, 30, 31] + [0, 1, 2, 3]
                   + [8, 9, 10, 11] + [16, 17, 18, 19])
        stage_a(6)
        attn_piece(3, [28, 29, 30, 31] + [0, 1, 2, 3] + [8, 9, 10, 11]
                   + [16, 17, 18, 19] + [24, 25, 26, 27])
        stage_a(1)
        attn_piece(1, [4, 5, 6, 7])
        attn_epi(1)
        attn_piece(2, [4, 5, 6, 7])
        attn_piece(3, [4, 5, 6, 7])
        stage_a(3)
        attn_piece(2, [12, 13, 14, 15])
        attn_epi(2)
        attn_piece(3, [12, 13, 14, 15])
        stage_a(5)
        attn_piece(3, [20, 21, 22, 23])
        attn_epi(3)

    nc.compile()
    return nc


def _get_program():
    global _PROGRAM
    if _PROGRAM is None:
        _PROGRAM = _build_program()
    return _PROGRAM


def _build_masks(p):
    m = np.zeros((4, KT, QB), np.float32)
    pk = np.arange(KT)[:, None]
    c = np.arange(QB)[None, :]
    for s in range(4):
        m[s] = (c >= pk + 128 * s).astype(np.float32)
    return m


def build_in_maps(inputs):
    x = np.asarray(inputs["x"], np.float32)
    wq = np.ascontiguousarray(np.asarray(inputs["Wq"], np.float32))
    wkv = np.ascontiguousarray(
        np.concatenate(
            [np.asarray(inputs["Wk"], np.float32),
             np.asarray(inputs["Wv"], np.float32)], axis=1
        )
    )
    bq = np.ascontiguousarray(np.asarray(inputs["bq"], np.float32)[:, None])
    bkv = np.ascontiguousarray(
        np.concatenate(
            [np.asarray(inputs["bk"], np.float32),
             np.asarray(inputs["bv"], np.float32)]
        )[:, None]
    )
    in_maps = []
    for core in range(NCORES):
        b, p = core // 2, core % 2
        xt = x[b].T
        if p:
            xt = np.roll(xt, -512, axis=1)
        in_maps.append({
            "xt": np.ascontiguousarray(xt),
            "wq": wq,
            "wkv": wkv,
            "bq": bq,
            "bkv": bkv,
            "masks": _build_masks(p),
            "ones": np.ones((128, NKT), np.float32),
            "tb": np.full((KT, 1), 0.0 if p == 1 else -1e30, np.float32),
        })
    return in_maps


def assemble_out(results):
    out = np.empty((B, T, H), np.float32)
    for core in range(NCORES):
        b, p = core // 2, core % 2
        o = np.asarray(results[core]["o"])
        for i in range(NB):
            g = 1024 * i + 512 * p
            out[b, g:g + QB] = o[i * QB:(i + 1) * QB]
    return out


def kernel(**inputs):
    from concourse.bass_utils import run_bass_kernel_spmd

    nc = _get_program()
    in_maps = build_in_maps(inputs)
    res = run_bass_kernel_spmd(nc, in_maps, list(range(NCORES)))
    return assemble_out(res.results)
